# revision 14
# baseline (speedup 1.0000x reference)
"""Multi-head attention (B=8, P=1024, D=768, H=12) on 8 TRN2 NeuronCores.

Strategy: pure data parallelism — batch element b runs on core b (no
collectives). Host pre-transposes x and casts operands to bf16; each core
computes QK^T/softmax/AV/proj for its batch element with all matmuls on the
TensorEngine (bf16, fp32 PSUM accumulation), exp on the ScalarEngine, and
evacuations/normalization on the VectorEngine.

v2 schedule: the attention inner loop is organized as 512-column units with
three dedicated PSUM pools (S: 2 banks, AV accumulators: 4 banks, utility:
2 banks). A filler stream of independent single-matmul closures (next pair's
QK^T feature tiles, then per-head normalization broadcasts) is interleaved
3 per k-tile so the PE stays busy while the ScalarEngine works through exp
— the exp pipeline (2.3us/k-tile) is slower than the S+AV matmuls
(1.7us/k-tile) and otherwise throttles the PE via PSUM recycling.

Self-contained: builds + compiles the Bass kernel on first call, runs via
PJRT (axon) across cores 0-7, and reassembles full outputs. Returns the
tuple (out, weighted_avg), matching the reference.
"""

import numpy as np
from collections import deque
from contextlib import ExitStack

import bass_rust
import concourse.bass as bass
import concourse.tile as tile
from concourse import mybir
from concourse import bass2jax

B, P, D = 8, 1024, 768
H = 12
HD = D // H            # 64
SCALE = HD ** -0.5     # 0.125
N_CORES = 8
KT = D // 128          # 6 contraction tiles over d
QT = P // 128          # 8 tiles over sequence
BF = mybir.dt.bfloat16
F32 = mybir.dt.float32
NP_BF16 = np.dtype(mybir.dt.np(BF))

IN_NAMES = ["xT", "wqk", "wv", "wph", "bqk", "bv", "bp", "ident", "selmat"]
OUT_NAMES = ["out", "wa"]


def _split_excess_waits(nc, max_waits=1):
    """This container's walrus build rejects instructions carrying more than
    one sync wait. Hoist excess waits onto same-engine no-ops inserted just
    before the overloaded instruction (engine queues execute in order, so
    wait-for-all-before-exec semantics are preserved)."""
    ctr = 0
    for bb in nc.main_func.blocks:
        newlist = []
        dirty = False
        for inst in bb.instructions:
            si = inst.sync_info
            waits = list(si.on_wait) if (si is not None and si.on_wait) else []
            if len(waits) > max_waits:
                excess, keep = waits[:-max_waits], waits[-max_waits:]
                for i in range(0, len(excess), max_waits):
                    chunk = excess[i : i + max_waits]
                    nop = bass_rust.InstNoOp(name=f"WSPILL-{ctr}")
                    ctr += 1
                    nop.engine = inst.engine
                    nop.sync_info = bass_rust.SyncInfo(on_wait=chunk, on_update=[])
                    newlist.append(nop)
                inst.sync_info = bass_rust.SyncInfo(
                    on_wait=keep, on_update=list(si.on_update or [])
                )
                dirty = True
            newlist.append(inst)
        if dirty:
            bb.instructions = newlist
    return ctr


def _bcast_ap(dram_ap, parts):
    """Partition-stride-0 DMA source view of a 1-D DRAM tensor: [n] -> [parts, n]."""
    return bass.AP(
        tensor=dram_ap.tensor,
        offset=dram_ap.offset,
        ap=[[0, parts]] + list(dram_ap.ap),
    )


def build_nc(split_waits=True, loop_n=None, unroll=1):
    nc = bass.Bass(target_bir_lowering=False)

    xT_e = nc.declare_dram_parameter("xT", [D, P], BF, isOutput=False)
    wqk_e = nc.declare_dram_parameter("wqk", [D, 2 * D], BF, isOutput=False)
    wv_e = nc.declare_dram_parameter("wv", [D, D], BF, isOutput=False)
    wph_e = nc.declare_dram_parameter("wph", [H // 2, 128, D], BF, isOutput=False)
    bqk_e = nc.declare_dram_parameter("bqk", [128, 2 * D // 128], F32, isOutput=False)
    bv_e = nc.declare_dram_parameter("bv", [D], F32, isOutput=False)
    bp_e = nc.declare_dram_parameter("bp", [D], F32, isOutput=False)
    id_e = nc.declare_dram_parameter("ident", [128, 128], BF, isOutput=False)
    sel_e = nc.declare_dram_parameter("selmat", [H, H * HD], BF, isOutput=False)
    out_e = nc.declare_dram_parameter("out", [P, D], BF, isOutput=True)
    wa_e = nc.declare_dram_parameter("wa", [P, D], BF, isOutput=True)

    EXP = mybir.ActivationFunctionType.Exp
    LN = mybir.ActivationFunctionType.Ln

    with tile.TileContext(nc) as tc, ExitStack() as ctx:
        if loop_n is not None:
            ctx.enter_context(tc.For_i(0, loop_n, 1))
        const = ctx.enter_context(tc.tile_pool(name="const", bufs=1))
        qkp = ctx.enter_context(tc.tile_pool(name="qkp", bufs=1))
        vxp = ctx.enter_context(tc.tile_pool(name="vxp", bufs=1))
        wtp = ctx.enter_context(tc.tile_pool(name="wtp", bufs=1))
        ptp = ctx.enter_context(tc.tile_pool(name="ptp", bufs=8))
        stgp = ctx.enter_context(tc.tile_pool(name="stgp", bufs=1))
        outp = ctx.enter_context(tc.tile_pool(name="outp", bufs=4))
        # PSUM: psS (S tiles, 2x1 bank) + psAV (AV accum, 2x2 banks) +
        # psX (qkT/vext/psr/proj utility, 2x1 bank) + psW (wa transpose,
        # 1x2 banks) = 8 banks.
        psS = ctx.enter_context(tc.tile_pool(name="psS", bufs=2, space="PSUM"))
        psAV = ctx.enter_context(tc.tile_pool(name="psAV", bufs=2, space="PSUM"))
        psX = ctx.enter_context(tc.tile_pool(name="psX", bufs=2, space="PSUM"))

        for _it in range(unroll):
            # ---- constant loads --------------------------------------------
            # xT/wqk double-buffered so the next unrolled iteration's loads
            # start immediately instead of waiting for this iteration's last
            # qkT reader.
            xT = [const.tile([128, P], BF, tag=f"xT{k}", name=f"xT{k}", bufs=2) for k in range(KT)]
            wqk = [const.tile([128, 2 * D], BF, tag=f"wqk{k}", name=f"wqk{k}", bufs=2) for k in range(KT)]
            wv = [const.tile([128, D], BF, tag=f"wv{k}", name=f"wv{k}") for k in range(KT)]
            wp = [const.tile([128, D], BF, tag=f"wp{p}", name=f"wp{p}") for p in range(H // 2)]
            bqk = const.tile([128, 2 * D // 128], F32, tag="bqk", name="bqk")
            bvb = const.tile([128, D], F32, tag="bvb", name="bvb")
            bpb = const.tile([128, D], F32, tag="bpb", name="bpb")
            ident = const.tile([128, 128], BF, tag="ident", name="ident")
            selmat = const.tile([H, H * HD], BF, tag="selmat", name="selmat")

            # DMA order = first-use order: x/qk weights and v weights lead;
            # proj/ident/selmat are tail-only and queue last.
            for k in range(KT):
                nc.sync.dma_start(out=xT[k], in_=xT_e[k * 128 : (k + 1) * 128, :])
                nc.sync.dma_start(out=wqk[k], in_=wqk_e[k * 128 : (k + 1) * 128, :])
                nc.gpsimd.dma_start(out=wv[k], in_=wv_e[k * 128 : (k + 1) * 128, :])
            nc.gpsimd.dma_start(out=bqk, in_=bqk_e[:])
            nc.gpsimd.dma_start(out=bvb, in_=_bcast_ap(bv_e[:], 128))
            nc.gpsimd.dma_start(out=selmat, in_=sel_e[:])
            for p in range(H // 2):
                nc.gpsimd.dma_start(out=wp[p], in_=wph_e[p])
            nc.gpsimd.dma_start(out=bpb, in_=_bcast_ap(bp_e[:], 128))
            nc.gpsimd.dma_start(out=ident, in_=id_e[:])

            # ---- phase 1: qT / kT = (w_qk)^T @ x^T  [feature-major] --------
            # qkT[m] rows = features m*128..; m 0..5 -> q, 6..11 -> k.
            # Pair pr's S needs tiles {pr, 6+pr}; the remaining tiles are
            # emitted via the filler stream inside the attention k-loop.
            qkT = [qkp.tile([128, P], BF, tag=f"qkT{m}", name=f"qkT{m}") for m in range(2 * D // 128)]

            def qkT_closures(ms):
                """One closure per matmul; the last of each (m, j) unit also
                emits the DVE bias-add evacuation into the qkT tile."""
                cls = []
                for m in ms:
                    for j in range(2):
                        st = {}

                        def mk(k, m=m, j=j, st=st):
                            def go():
                                if k == 0:
                                    st["ps"] = psX.tile([128, 512], F32, tag="px", name="px")
                                nc.tensor.matmul(
                                    st["ps"],
                                    lhsT=wqk[k][:, m * 128 : (m + 1) * 128],
                                    rhs=xT[k][:, j * 512 : (j + 1) * 512],
                                    start=(k == 0),
                                    stop=(k == KT - 1),
                                )
                                if k == KT - 1:
                                    nc.vector.tensor_scalar_add(
                                        qkT[m][:, j * 512 : (j + 1) * 512],
                                        st["ps"],
                                        bqk[:, m : m + 1],
                                    )
                            return go

                        cls.extend(mk(k) for k in range(KT))
                return cls

            for cl in qkT_closures([0, 6]):
                cl()

            # ---- phase 2: v natural [seq-major] with ones column ------------
            # vext[p][:, h, 0:64] = v_h rows p*128..; vext[p][:, h, 64] = 1.0
            vext = [vxp.tile([128, H, HD + 1], BF, tag=f"vext{p}", name=f"vext{p}") for p in range(QT)]
            for p in range(QT):
                nc.vector.memset(vext[p][:, :, HD : HD + 1], 1.0)
                for (c0, cw) in ((0, 512), (512, 256)):
                    ps = psX.tile([128, 512], F32, tag="px", name="px")
                    for k in range(KT):
                        nc.tensor.matmul(
                            ps[:, :cw],
                            lhsT=xT[k][:, p * 128 : (p + 1) * 128],
                            rhs=wv[k][:, c0 : c0 + cw],
                            start=(k == 0),
                            stop=(k == KT - 1),
                        )
                    nh = cw // HD
                    nc.vector.tensor_add(
                        vext[p][:, c0 // HD : c0 // HD + nh, 0:HD],
                        ps[:, :cw].rearrange("p (h d) -> p h d", d=HD),
                        bvb[:, c0 : c0 + cw].rearrange("p (h d) -> p h d", d=HD),
                    )

            # ---- phase 3: attention per head pair ---------------------------
            # S^T units [128 kpos, 512 q]: lhsT = kT slice (64 feats, even head
            # at PE rows 0-63, odd at 64-127), rhs = qT. exp on ScalarE (scale
            # folded) per 512-unit. AV (lhsT=[v_h | ones], psum row 64 = softmax
            # denominator) lags the S/exp pipeline by one k-tile. Filler
            # closures keep the PE fed while ScalarE catches up.
            waTp = [wtp.tile([128, P], BF, tag=f"waTp{p}", name=f"waTp{p}") for p in range(H // 2)]
            dens12 = stgp.tile([H, P], BF, tag="dens12", name="dens12")
            recip12 = stgp.tile([H, P], F32, tag="recip12", name="recip12")
            recip12b = stgp.tile([H, P], BF, tag="recip12b", name="recip12b")
            nc.vector.memset(dens12, 1.0)
            stg_tiles = {}

            def norm_closures(heads, pool=None, tag="px"):
                """Per (h, j): recip broadcast via selector matmul, then DVE
                multiply of the staged AV rows; odd heads merge into the pair
                tile's upper partitions by DMA."""
                cls = []
                for h in heads:
                    for j in range(2):
                        def go(h=h, j=j):
                            psr = (pool or psX).tile([HD, 512], F32, tag=tag, name="psr")
                            nc.tensor.matmul(
                                psr,
                                lhsT=selmat[:, h * HD : (h + 1) * HD],
                                rhs=recip12b[0:H, j * 512 : (j + 1) * 512],
                                start=True,
                                stop=True,
                            )
                            sl = slice(j * 512, (j + 1) * 512)
                            if h % 2 == 0:
                                nc.vector.tensor_mul(
                                    waTp[h // 2][0:HD, sl], stg_tiles[h][0:HD, sl], psr
                                )
                            else:
                                wt = outp.tile([HD, 512], BF, tag="wtmp", name="wtmp")
                                nc.vector.tensor_mul(wt, stg_tiles[h][0:HD, sl], psr)
                                nc.sync.dma_start(out=waTp[h // 2][HD:128, sl], in_=wt)
                        cls.append(go)
                return cls

            filler = deque()
            for pr in range(H // 2):
                heads = (2 * pr, 2 * pr + 1)
                if pr + 1 < H // 2:
                    filler.extend(qkT_closures([pr + 1, 6 + pr + 1]))
                if pr == H // 2 - 1:
                    filler.extend(norm_closures(range(0, 10)))
                psav = {h: psAV.tile([HD + 1, P], F32, tag="av", name="av") for h in heads}
                pt_prev = None
                for kt in range(QT + 1):
                    cur = {}
                    for hi, h in enumerate(heads):
                        if kt < QT:
                            base = (h % 2) * 64
                            pss = {}
                            for j in range(2):
                                pss[j] = psS.tile([128, 512], F32, tag="ss", name="ss")
                                nc.tensor.matmul(
                                    pss[j],
                                    lhsT=qkT[6 + h // 2][base : base + 64, kt * 128 : (kt + 1) * 128],
                                    rhs=qkT[h // 2][base : base + 64, j * 512 : (j + 1) * 512],
                                    start=True,
                                    stop=True,
                                )
                        if hi == 0 and kt > 0:  # AV for previous k-tile
                            for h2 in heads:
                                for j in range(2):
                                    nc.tensor.matmul(
                                        psav[h2][:, j * 512 : (j + 1) * 512],
                                        lhsT=vext[kt - 1][:, h2, :],
                                        rhs=pt_prev[h2][j],
                                        start=(kt - 1 == 0),
                                        stop=(kt - 1 == QT - 1),
                                    )
                        if kt < QT:
                            cur[h] = {}
                            for j in range(2):
                                pt = ptp.tile([128, 512], BF, tag="pt", name="pt")
                                nc.scalar.activation(pt, pss[j], EXP, scale=SCALE)
                                cur[h][j] = pt
                    pt_prev = cur
                    # mildly front-loaded filler pacing: the next pair's qkT
                    # tiles land before its first S reads them, while late
                    # k-tiles keep enough filler to cover the exp lag
                    for _ in range((4, 4, 3, 3, 3, 3, 2, 2, 0)[kt]):
                        if filler:
                            filler.popleft()()
                for h in heads:
                    stg = stgp.tile([HD + 1, P], BF, tag=f"stg{h}", name=f"stg{h}")
                    nc.vector.tensor_copy(stg, psav[h])
                    nc.gpsimd.dma_start(out=dens12[h : h + 1, :], in_=stg[HD : HD + 1, :])
                    stg_tiles[h] = stg
                if pr in (3, 4, 5):
                    # reciprocal of denominator rows as 1/d = exp(-ln(d)) on
                    # the ScalarEngine (the natural_log set contains both)
                    nc.scalar.activation(recip12[0:12, :], dens12[0:12, :], LN)
                    nc.scalar.activation(recip12b[0:12, :], recip12[0:12, :], EXP, scale=-1.0)

            # drain remaining filler (normally just pair-5 leftovers)
            while filler:
                filler.popleft()()

            # ---- phase 4: per q-tile, wa transposes fused with proj --------
            # qt=0 is split around the last pair's normalization (heads 10/11
            # wait on the pair-5 recip chain): its pair-0..4 transposes and
            # proj partial-accumulations run first so the PE stays busy while
            # the chain resolves; the pair-5 pieces close the groups after.
            def emit_transposes(qt, prs, psw):
                for p in prs:
                    nc.tensor.matmul(
                        psw[:, p * 128 : (p + 1) * 128],
                        lhsT=waTp[p][:, qt * 128 : (qt + 1) * 128],
                        rhs=ident,
                        start=True,
                        stop=True,
                    )

            def emit_proj(qt, ps, c0, cw, prs, start, stop):
                for i, p in enumerate(prs):
                    nc.tensor.matmul(
                        ps[:, :cw],
                        lhsT=waTp[p][:, qt * 128 : (qt + 1) * 128],
                        rhs=wp[p][:, c0 : c0 + cw],
                        start=start and i == 0,
                        stop=stop and i == len(prs) - 1,
                        skip_group_check=True,
                    )

            def emit_evacs(qt, psw, pss):
                wa_sb = outp.tile([128, D], BF, tag="wa_sb", name="wa_sb")
                nc.scalar.copy(wa_sb, psw)
                nc.gpsimd.dma_start(out=wa_e[qt * 128 : (qt + 1) * 128, :], in_=wa_sb)
                out_sb = outp.tile([128, D], BF, tag="out_sb", name="out_sb")
                for (c0, cw), ps in pss:
                    nc.vector.tensor_add(
                        out_sb[:, c0 : c0 + cw], ps[:, :cw], bpb[:, c0 : c0 + cw]
                    )
                nc.sync.dma_start(out=out_e[qt * 128 : (qt + 1) * 128, :], in_=out_sb)

            psw0 = psAV.tile([128, D], F32, tag="av", name="psw")
            emit_transposes(0, range(5), psw0)
            pss0 = []
            for (c0, cw) in ((0, 512), (512, 256)):
                ps = psS.tile([128, 512], F32, tag="ss", name="ss")
                emit_proj(0, ps, c0, cw, range(5), True, False)
                pss0.append(((c0, cw), ps))
            for cl in norm_closures(range(10, H)):
                cl()
            emit_transposes(0, [5], psw0)
            for (c0, cw), ps in pss0:
                emit_proj(0, ps, c0, cw, [5], False, True)
            emit_evacs(0, psw0, pss0)

            for qt in range(1, QT):
                psw = psAV.tile([128, D], F32, tag="av", name="psw")
                emit_transposes(qt, range(H // 2), psw)
                pss = []
                for (c0, cw) in ((0, 512), (512, 256)):
                    ps = psS.tile([128, 512], F32, tag="ss", name="ss")
                    emit_proj(qt, ps, c0, cw, range(H // 2), True, True)
                    pss.append(((c0, cw), ps))
                emit_evacs(qt, psw, pss)

    if split_waits:
        _split_excess_waits(nc)
    return nc


def make_in_maps(x, w_qkv, b_qkv, w_proj, b_proj):
    """Host-side shard prep: batch element b -> core b; weights replicated."""
    xf = np.asarray(x, dtype=np.float32)
    wqkv = np.asarray(w_qkv, dtype=np.float32)
    bqkv = np.asarray(b_qkv, dtype=np.float32)
    wproj = np.asarray(w_proj, dtype=np.float32)
    bproj = np.asarray(b_proj, dtype=np.float32)

    wqk = np.ascontiguousarray(wqkv[:, : 2 * D]).astype(NP_BF16)
    wv = np.ascontiguousarray(wqkv[:, 2 * D :]).astype(NP_BF16)
    wph = np.ascontiguousarray(wproj.reshape(H // 2, 128, D)).astype(NP_BF16)
    bqk = np.ascontiguousarray(bqkv[: 2 * D].reshape(2 * D // 128, 128).T)
    bv = np.ascontiguousarray(bqkv[2 * D :])
    ident = np.eye(128, dtype=np.float32).astype(NP_BF16)
    selmat = np.kron(np.eye(H, dtype=np.float32), np.ones((1, HD), np.float32)).astype(NP_BF16)

    in_maps = []
    for b in range(N_CORES):
        in_maps.append(
            {
                "xT": np.ascontiguousarray(xf[b].T).astype(NP_BF16),
                "wqk": wqk,
                "wv": wv,
                "wph": wph,
                "bqk": bqk,
                "bv": bv,
                "bp": bproj,
                "ident": ident,
                "selmat": selmat,
            }
        )
    return in_maps


_CACHE = {}


def _get_nc():
    if "nc" not in _CACHE:
        _CACHE["nc"] = build_nc()
    return _CACHE["nc"]


def run_once(in_maps, nc=None):
    """One 8-core execution via the PJRT redirect path (fresh jit per call;
    NEFF comes from the neuron compile cache after the first call)."""
    if nc is None:
        nc = _get_nc()
    return bass2jax.run_bass_via_pjrt(nc, in_maps, n_cores=N_CORES)


def kernel(x, w_qkv, b_qkv, w_proj, b_proj):
    in_maps = make_in_maps(x, w_qkv, b_qkv, w_proj, b_proj)
    results = run_once(in_maps)
    out = np.stack([results[b]["out"] for b in range(N_CORES)]).astype(np.float32)
    wa = np.stack([results[b]["wa"] for b in range(N_CORES)]).astype(np.float32)
    return (out, wa)
